# revision 39
# baseline (speedup 1.0000x reference)
"""Trainium2 Bass kernel for nn_DecoderBlock (masked self-attn + cross-attn + FFN).

Strategy: pure data-parallel over batch. B=64 batches are split 8 per core
across the 8 NeuronCores; each core runs an identical (SPMD) Bass program on
its shard with the full weight set replicated. No collectives needed.

Per-core program: batch items are processed in PAIRS so that every matmul
whose stationary operand is a shared weight runs with a 512-wide moving
operand, and every ScalarE / DVE op covers [128, 512] tiles.

fp8 DoubleRow datapath (the big win over the bf16 version, ~1.8x measured
matmul throughput): every 512-or-deeper contraction (QKV / cross-KV/Q /
out-projections / both FFN matmuls) runs as dual-fp8 DoubleRow matmuls that
contract 256 rows per pass.  Weights are pre-scaled by SW=32 (so N(0,0.02)
entries land in e4m3's normal range), clipped to +-240 (TRN fp8e4 tops out
there) and packed [K//256, 128, 2, N] on the HOST; activations are evicted
to fp8 packs by the ScalarE/DVE.  Residuals enter the out-proj/FFN PSUMs
through a 1024*I bf16 identity so they ride at the same SW^2 scale as the
fp8 products, and every block boundary is a LayerNorm, which is
scale-invariant -- so all the scaling cancels exactly (exp gets
scale=0.125/1024 to undo SW^2 on the scores).  HW ISA restrictions found on
the way: dual-fp8 LDWEIGHTS rejects stationary tiles narrower than ~64
(hence sums selectors are [128,2,64], writing a [64,W] sums PSUM), dual-fp8
matmuls require dst partition base 0 (odd heads' PV falls back to plain
fp8 matmuls at tile_position (0,64)), and GPSIMD cannot read PSUM.

Softmax (transposed scores, no max-subtraction -- scores provably bounded;
p = exp(s) <= ~5 so fp8's 240 max is safe):
  - the causal mask costs ZERO PE work: exp writes the fp8 softmax
    numerators, then a Pool affine_select zeroes the upper triangle of the
    diagonal [128,128] blocks in place;
  - self-attn numerators live in per-head persistent [128, 2(kb), 512]
    fp8 packs whose kb1 dead columns stay zero forever, so ONE DoubleRow
    matmul contracts all 256 keys for sums and (even-head) PV;
  - per-query sums ride selector matmuls into [64,W] PSUM -> one DVE
    reciprocal_approx_fast per 4 heads; 1/sum is partition-broadcast by one
    PE matmul per head-pair and folded into the A^T eviction as one DVE
    multiply writing the fp8 out-proj stationary packs directly.

Software pipeline: pair p's FFN (22 chunks) is deferred and emitted as PE
filler inside pair p+1's attention-output LayerNorm windows; pair p+1's
stageA fills the second window.  PSUM: S(2)+AB(2)+sums(2)+work(2) = 8 banks.

LayerNorm: bn_stats/bn_aggr on DVE; rstd = 1/sqrt(var+eps) via the bit-trick
seed plus one Newton step, entirely on the DVE -- the scalar engine keeps
ONE activation table for the whole kernel.  The normalize is fused into the
PSUM->SBUF eviction (3 of 4 on ScalarE, 1 on DVE to balance engines).

Host prep (prep_args): f32->bf16 for x/enc_out, f32->scaled-fp8-packed for
the seven projection/FFN weight matrices, biases pre-scaled by SW or SW^2 to
match the PSUM scale they join.  Baseline (pure bf16): 501991 ns HW; this
version: ~400000 ns HW (slope-measured), rel err 1.72e-2 (< 2e-2 gate).
"""

import numpy as np
import ml_dtypes
from contextlib import ExitStack

import concourse.bass as bass
import concourse.bacc as bacc
import concourse.tile as tile
from concourse import mybir, masks
from concourse.bass_utils import run_bass_kernel_spmd

E, H, D, HD = 512, 8, 64, 512
T = 256
B_FULL = 64
N_CORES = 8
BL = B_FULL // N_CORES
P = 128
W = 512          # pair-tile free width (2 batch items x T columns)
F32 = mybir.dt.float32
F32R = mybir.dt.float32r
BF16 = mybir.dt.bfloat16
FP8 = mybir.dt.float8e4
I32 = mybir.dt.int32
AF = mybir.ActivationFunctionType
ALU = mybir.AluOpType
DR = mybir.MatmulPerfMode.DoubleRow
EPS = 1e-5
SW = 32.0               # fp8 weight pre-scale (host); LN absorbs SW^2 per block
EXPS = 0.125 / (SW * SW)  # softmax exp scale compensating SW^2 on the scores
FP8_NP = mybir.dt.np(FP8)

WEIGHT_NAMES = [
    'mq_w', 'mk_w', 'mv_w', 'mproj_w', 'mproj_b',
    'cq_w', 'cq_b', 'ck_w', 'ck_b', 'cv_w', 'cv_b', 'co_w', 'co_b',
    'f1_w', 'f1_b', 'f2_w', 'f2_b',
    'ln1_s', 'ln1_b', 'ln2_s', 'ln2_b', 'ln3_s', 'ln3_b',
]


def build_program(n_batch=BL, apply_ln_sb=False, apply_bias=False, n_rep=1):
    nc = bacc.Bacc("TRN2", target_bir_lowering=False, debug=False)

    io = {}
    io['x'] = nc.dram_tensor('x', [n_batch, T, E], BF16, kind="ExternalInput").ap()
    io['enc_out'] = nc.dram_tensor('enc_out', [n_batch, T, E], BF16, kind="ExternalInput").ap()
    for name in WEIGHT_NAMES:
        if name.endswith('_w'):
            # host packs 2D weights for fp8 DoubleRow: [K//256, 128, 2, N]
            K, N = (4 * E, E) if name == 'f2_w' else (E, 4 * E if name == 'f1_w' else E)
            shape = [K // 256, P, 2, N]
            dt = FP8
        elif name == 'f1_b':
            shape, dt = [4 * E], F32
        else:
            shape, dt = [E], F32
        io[name] = nc.dram_tensor(name, shape, dt, kind="ExternalInput").ap()
    io['out'] = nc.dram_tensor('out', [n_batch, T, E], F32, kind="ExternalOutput").ap()

    with tile.TileContext(nc) as tc:
        with ExitStack() as ctx:
            _emit(ctx, tc, io, n_batch, apply_ln_sb, apply_bias, n_rep)
    nc.compile()
    return nc


def _emit(ctx, tc, io, n_batch, apply_ln_sb, apply_bias, n_rep=1):
    nc = tc.nc
    n_pair = n_batch // 2

    wpool = ctx.enter_context(tc.tile_pool(name="weights", bufs=1))
    const = ctx.enter_context(tc.tile_pool(name="const", bufs=1))
    anat = ctx.enter_context(tc.tile_pool(name="anat", bufs=2))
    atrn = ctx.enter_context(tc.tile_pool(name="atrn", bufs=2))
    attn = ctx.enter_context(tc.tile_pool(name="attn", bufs=2))
    small = ctx.enter_context(tc.tile_pool(name="small", bufs=2))
    # PSUM: S(2) + AB(2) + sums0(1) + sums1(1) + work(2) = 8 banks
    psS = ctx.enter_context(tc.tile_pool(name="psS", bufs=2, space="PSUM"))
    psAB = ctx.enter_context(tc.tile_pool(name="psAB", bufs=2, space="PSUM"))
    psSum = ctx.enter_context(tc.tile_pool(name="psSum", bufs=1, space="PSUM"))
    psW = ctx.enter_context(tc.tile_pool(name="psW", bufs=2, space="PSUM"))

    # ---- constants ----
    ident = const.tile([P, P], F32)
    masks.make_identity(nc, ident[:])
    ident_b = const.tile([P, P], BF16)
    nc.vector.tensor_copy(ident_b[:], ident[:])
    # scaled identity: residual enters out-proj / FFN PSUMs at the SW^2 scale
    # of the fp8 products (LN downstream is scale-invariant)
    ident_sc = const.tile([P, P], BF16)
    nc.vector.tensor_scalar_mul(ident_sc[:], ident[:], SW * SW)
    ones_row_f = const.tile([1, P], F32)
    nc.vector.memset(ones_row_f[:], 1.0)
    ones_row = const.tile([1, P], F32R)
    nc.vector.tensor_copy(ones_row[:], ones_row_f[:])
    # sel64[h][:, :, h] = 1 else 0 (both DR halves): routes a head's
    # column-sums into row h of a [64, W] PSUM tile via one DoubleRow matmul
    # (M=64 because dual-fp8 LDWEIGHTS rejects tiny stationary tiles).
    sel64 = []
    for hh in range(4):
        t = const.tile([P, 2, 64], FP8, tag=f"sel64_{hh}")
        nc.vector.memset(t[:], 0.0)
        nc.vector.memset(t[:, :, hh:hh + 1], 1.0)
        sel64.append(t)
    # selp[i] [4, 128]: cols 0:64 pick row 2i, cols 64:128 pick row 2i+1 --
    # one matmul broadcasts two heads' 1/sums rows to the 128 A^T partitions.
    selp = []
    for i in range(2):
        tf = const.tile([4, P], F32, tag=f"selpf_{i}")
        nc.gpsimd.memset(tf[:], 1.0)
        # keep where partition == 2i + (col // 64)
        nc.gpsimd.affine_select(out=tf[:], in_=tf[:],
                                compare_op=ALU.is_equal, fill=0.0, base=2 * i,
                                pattern=[[1, 2], [0, 64]], channel_multiplier=-1)
        t = const.tile([4, P], BF16, tag=f"selp_{i}")
        nc.vector.tensor_copy(t[:], tf[:])
        selp.append(t)

    def emit_dma_in(p):
        # inputs are bf16 in DRAM (host pre-cast); issue on SP's HWDGE to
        # keep the Pool engine free
        xs, es = [], []
        for j in range(4):
            b, th = 2 * p + j // 2, j % 2
            xt = anat.tile([P, W], BF16, tag="x_nat", bufs=8, name="x_nat")
            nc.sync.dma_start(out=xt[:], in_=io['x'][b, th * P:(th + 1) * P, :])
            et = anat.tile([P, W], BF16, tag="enc_nat", bufs=4, name="enc_nat")
            nc.sync.dma_start(out=et[:], in_=io['enc_out'][b, th * P:(th + 1) * P, :])
            xs.append(xt)
            es.append(et)
        return xs, es

    # ---- weights resident in SBUF as fp8 DoubleRow packs [P, 2, n] ----
    def load_dr(name, n):
        ts = []
        for g in range(io[name].shape[0]):
            t = wpool.tile([P, 2, n], FP8, tag=f"w_{name}_{g}", name="wdr")
            nc.gpsimd.dma_start(out=t[:], in_=io[name][g])
            ts.append(t)
        return ts

    dma0 = emit_dma_in(0)

    mqw = load_dr('mq_w', HD)
    mkw = load_dr('mk_w', HD)
    mvw = load_dr('mv_w', HD)
    ckw = load_dr('ck_w', HD)
    cvw = load_dr('cv_w', HD)
    # pair-1 inputs issue ahead of the late-needed weights (mp/cq/co/f1/f2
    # aren't read until much later)
    dma1 = emit_dma_in(1) if n_batch > 2 else None
    mpw = load_dr('mproj_w', E)
    cqw = load_dr('cq_w', HD)
    cow = load_dr('co_w', E)
    f1sb = load_dr('f1_w', 4 * E)          # 2 x [128, 2, 2048]
    f2sb = load_dr('f2_w', E)              # 8 x [128, 2, 512]

    # f1 bias as per-partition columns [P, 16]
    f1b_col = const.tile([P, 16], F32)
    for j in range(16):
        nc.gpsimd.dma_start(out=f1b_col[:, j:j + 1], in_=io['f1_b'][j * P:(j + 1) * P][:, None])

    # persistent fp8 softmax-numerator tiles for self-attn, [key_p, kb, q]:
    # kb0 half fully rewritten each pair; kb1 half only live query cols (the
    # dead cols stay zero forever so one DoubleRow matmul covers both blocks)
    pp_self = []
    for h in range(H):
        t = attn.tile([P, 2, W], FP8, tag=f"pp_{h}", bufs=1, name="pp")
        nc.vector.memset(t[:], 0.0)
        pp_self.append(t)

    if apply_bias:
        bias_rows = {}
        for nm in ('mproj_b', 'cv_b', 'co_b', 'f2_b'):
            t = const.tile([1, E], F32R, tag=f"br_{nm}")
            nc.gpsimd.dma_start(out=t[:1, :], in_=io[nm][None, :])
            bias_rows[nm] = t
        bias_cols = {}
        for nm in ('cq_b', 'ck_b'):
            t = const.tile([P, 4], F32, tag=f"bc_{nm}")
            for j in range(4):
                nc.gpsimd.dma_start(out=t[:, j:j + 1], in_=io[nm][j * P:(j + 1) * P][:, None])
            bias_cols[nm] = t

    if apply_ln_sb:
        ln_bc = {}
        for nm in ('ln1_s', 'ln1_b', 'ln2_s', 'ln2_b', 'ln3_s', 'ln3_b'):
            t = const.tile([P, E], F32, tag=f"ln_{nm}")
            src_ap = io[nm]
            bc = bass.AP(tensor=src_ap.tensor, offset=src_ap.offset,
                         ap=[[0, P]] + list(src_ap.ap))
            nc.sync.dma_start(out=t[:], in_=bc)
            ln_bc[nm] = t

    # alternating eviction engine (balance ScalarE / DVE; Pool cannot read
    # PSUM on real HW)
    ev_state = {'i': 0}

    def evict(dst, src):
        ev_state['i'] += 1
        if ev_state['i'] % 2 == 0:
            nc.scalar.activation(dst, src, AF.Copy)
        else:
            nc.vector.tensor_copy(dst, src)

    # ---- building blocks ----
    def transpose4(srcs, tag, nb, pull=None):
        """srcs: 4 natural [P, W] bf16 tiles -> 2 fp8 DR packs [P, 2, W]
        ([e_chunk_pair, half] layout: half i holds e-chunk 2a+i transposed)."""
        outs = []
        for a in range(2):
            o = atrn.tile([P, 2, W], FP8, tag=tag, bufs=nb, name="trdr")
            for half in range(2):
                if pull is not None:
                    pull(1)
                eb = 2 * a + half
                ps = psAB.tile([P, W], BF16, tag="AB", name="ps_tr")
                for j in range(4):
                    nc.tensor.transpose(ps[:, j * P:(j + 1) * P],
                                        srcs[j][:, eb * P:(eb + 1) * P], ident_b[:])
                evict(o[:, half, :], ps[:])
            outs.append(o)
        return outs

    def proj_T(wdr, srcT8, tag, nb, bias_col=None, pool=None, pull=None):
        """out[m][hd_p, pair_t] = SW*(W^T x^T); 4 x [P, W] bf16."""
        outs = []
        for m in range(4):
            if pull is not None:
                pull(1)
            pl, ptag = pool or (psW, "work")
            ps = pl.tile([P, W], F32, tag=ptag, name="ps_p")
            for g in range(2):
                nc.tensor.matmul(ps[:], wdr[g][:, :, m * P:(m + 1) * P], srcT8[g][:],
                                 start=(g == 0), stop=(g == 1), perf_mode=DR)
            o = atrn.tile([P, W], BF16, tag=tag, bufs=nb, name="projt")
            if bias_col is not None:
                nc.vector.tensor_scalar_add(o[:], ps[:], bias_col[:, m:m + 1])
            else:
                evict(o[:], ps[:])
            outs.append(o)
        return outs

    def proj_N(wdr, srcT8, tag, nb, bias_row=None):
        """value proj, fp8 DR-packed per batch item: out[b][key_p, kb, hd]."""
        outs = []
        for b in range(2):
            o = anat.tile([P, 2, W], FP8, tag=tag, bufs=nb, name="vdr")
            for kb in range(2):
                j = b * 2 + kb
                ps = psW.tile([P, W], F32, tag="work", name="ps_v")
                for g in range(2):
                    nc.tensor.matmul(ps[:], srcT8[g][:, :, j * P:(j + 1) * P],
                                     wdr[g][:],
                                     start=(g == 0),
                                     stop=(g == 1) and bias_row is None,
                                     perf_mode=DR)
                if bias_row is not None:
                    nc.tensor.matmul(ps[:], ones_row[:1, :], bias_row[:1, :],
                                     start=False, stop=True)
                evict(o[:, kb, :], ps[:])
            outs.append(o)
        return outs

    def attention(QT, KT, Vn, is_causal, p_tag, pull=None):
        """QT/KT: 4 x [P(hd), W(pair_t)] bf16 (at SW scale); Vn: 2 x
        [P(key), 2(kb), W(hd)] fp8. Returns 2 x [P, 2, W] fp8 A^T DR packs
        (each half = one head-pair's A^T at SW scale)."""
        pull = pull or (lambda n=1: None)
        ATs = [None] * 2
        A_tiles = [None] * 4
        sums_ps = [psSum.tile([64, W], F32, tag=f"sums{i}", bufs=1, name="sums")
                   for i in range(2)]
        rsb = [None, None]
        p_of = {}

        def tri_mask(view):
            # zero the upper triangle (query col < key partition) in place on
            # the Pool engine: keeps the mask off the PE entirely
            nc.gpsimd.affine_select(out=view, in_=view, compare_op=ALU.is_ge,
                                    fill=0.0, base=0,
                                    pattern=[[0, 2], [1, P]],
                                    channel_multiplier=-1)

        def emit_S_exp(h):
            m, r = h // 2, (h % 2) * 64
            if is_causal:
                pp = pp_self[h]
            else:
                pp = attn.tile([P, 2, W], FP8, tag=p_tag, bufs=4, name="pp")
            for kb in range(2):
                S = psS.tile([P, W], F32, tag="S", name="S")
                live = is_causal and kb == 1  # queries 0:127 fully masked
                for b in range(2):
                    ks = KT[m][r:r + 64, b * T + kb * P: b * T + (kb + 1) * P]
                    q0 = b * T + (P if live else 0)
                    qs = QT[m][r:r + 64, q0:(b + 1) * T]
                    nc.tensor.matmul(S[:, q0:(b + 1) * T], ks, qs,
                                     start=True, stop=True,
                                     skip_group_check=True)
                if live:
                    # only live query columns (t 128:256 of each batch item)
                    src = S[:].rearrange("p (b t) -> p b t", b=2)[:, :, P:2 * P]
                    dst = pp[:, 1, :].rearrange("p (b t) -> p b t", b=2)[:, :, P:2 * P]
                    nc.scalar.activation(dst, src, AF.Exp, scale=EXPS)
                    tri_mask(dst)
                else:
                    nc.scalar.activation(pp[:, kb, :], S[:], AF.Exp, scale=EXPS)
                    if is_causal:  # kb0: triangle on the diagonal [P,P] blocks
                        pb = pp[:, 0, :].rearrange("p (b t) -> p b t", b=2)[:, :, 0:P]
                        tri_mask(pb)
            p_of[h] = pp

        def emit_sums_pv(h):
            g = h // 2
            r = (h % 2) * 64
            sp = sums_ps[h // 4]
            hr = h % 4
            pp = p_of[h]
            nc.tensor.matmul(sp[:], sel64[hr][:], pp[:],
                             start=(hr == 0), stop=(hr == 3),
                             perf_mode=DR, skip_group_check=True)
            if h % 2 == 0:
                A_tiles[g] = psAB.tile([P, W], F32, tag="AB", name="A_ps")
            A = A_tiles[g]
            ppb = pp[:].rearrange("p i (b t) -> p i b t", b=2)
            for b in range(2):
                if r == 0:
                    # dual-fp8 matmul requires dst partition base 0
                    nc.tensor.matmul(A[0:64, b * T:(b + 1) * T],
                                     Vn[b][:, :, h * 64:(h + 1) * 64],
                                     ppb[:, :, b, :],
                                     start=True, stop=True, perf_mode=DR,
                                     tile_position=(0, 0),
                                     skip_group_check=True)
                else:
                    # odd head rows 64:128: plain fp8 matmuls per key block
                    for kb in range(2):
                        q0 = b * T + (P if (is_causal and kb == 1) else 0)
                        nc.tensor.matmul(A[64:128, q0:(b + 1) * T],
                                         Vn[b][:, kb, h * 64:(h + 1) * 64],
                                         pp[:, kb, q0:(b + 1) * T],
                                         start=(kb == 0), stop=(kb == 1),
                                         tile_position=(0, 64),
                                         skip_group_check=True)

        def emit_recip(i):
            tf = attn.tile([4, W], F32, tag="rsbf", bufs=2, name="rsbf")
            nc.vector.reciprocal_approx_fast(tf[:], sums_ps[i][0:4, :])
            t = attn.tile([4, W], BF16, tag="rsb", bufs=2, name="rsb")
            nc.gpsimd.tensor_copy(t[:], tf[:])
            rsb[i] = t

        def emit_bc_at(g):
            bc = psW.tile([P, W], F32, tag="work", name="bc_ps")
            nc.tensor.matmul(bc[:], selp[g % 2][:], rsb[g // 2][:],
                             start=True, stop=True, skip_group_check=True)
            bc_sb = attn.tile([P, W], BF16, tag="bcsb", bufs=2, name="bc_sb")
            evict(bc_sb[:], bc[:])
            if g % 2 == 0:
                ATs[g // 2] = atrn.tile([P, 2, W], FP8, tag="at", bufs=4,
                                        name="at")
            nc.vector.tensor_mul(ATs[g // 2][:, g % 2, :], A_tiles[g][:],
                                 bc_sb[:])

        # Emission order keeps PE streaming and avoids ring-buffer deadlock:
        # groups 0/1 are normalized (bc+at) before A-tile slots are reused by
        # groups 2/3.
        for h in range(H):
            emit_S_exp(h)
            if h == 5:
                emit_bc_at(0)
                emit_bc_at(1)
            if h >= 1:
                emit_sums_pv(h - 1)
            if h == 4:
                emit_recip(0)
        emit_sums_pv(7)
        emit_recip(1)
        emit_bc_at(2)
        emit_bc_at(3)
        return ATs

    # LayerNorm helpers ------------------------------------------------
    def ln_stats(y_ps, mvall, jj):
        stats = small.tile([P, 6], F32, tag="bnst", bufs=4, name="stats")
        nc.vector.bn_stats(stats[:], y_ps[:])
        nc.vector.bn_aggr(mvall[:, 2 * jj:2 * jj + 2], stats[:])

    MAGIC2 = 0x5F3759DF + 0x80000000 + 1 - (1 << 32)  # magic + (~u>>1) carry fix

    def ln_rstd(mvall, n):
        """mvall [P,2n] = (m0,v0,..) -> rstd [P,n], nmr [P,n] = -m*rstd.

        rstd = 1/sqrt(var+eps) via the bit-trick seed + Newton iteration,
        entirely on the DVE -- keeps the scalar engine on one act table."""
        mv3 = mvall[:].rearrange("p (j two) -> p j two", two=2)
        var_ap = mv3[:, :, 1:2]
        mean_ap = mv3[:, :, 0:1]
        veps = small.tile([P, n], F32, tag=f"veps{n}", bufs=4, name="veps")
        nc.vector.tensor_scalar_add(veps[:], var_ap, EPS)
        u = veps[:].bitcast(I32)
        nt = small.tile([P, n], F32, tag=f"ntmp{n}", bufs=4, name="ntmp")
        nc.vector.tensor_tensor(nt[:].bitcast(I32), u, u, op=ALU.bitwise_not)
        y = small.tile([P, n], F32, tag=f"yseed{n}", bufs=4, name="yseed")
        nc.vector.tensor_scalar(y[:].bitcast(I32), nt[:].bitcast(I32), 1, None,
                                op0=ALU.logical_shift_right)
        nc.vector.tensor_scalar_add(y[:].bitcast(I32), y[:].bitcast(I32), MAGIC2)
        rstd = y
        for it in range(1):
            t1 = small.tile([P, n], F32, tag=f"nr{it}a{n}", bufs=4, name="nra")
            nc.vector.tensor_mul(t1[:], rstd[:], rstd[:])
            nc.vector.tensor_mul(t1[:], t1[:], veps[:])
            nc.vector.tensor_scalar(t1[:], t1[:], -0.5, 1.5, op0=ALU.mult, op1=ALU.add)
            y2 = small.tile([P, n], F32, tag=f"nr{it}b{n}", bufs=4, name="nrb")
            nc.vector.tensor_mul(y2[:], rstd[:], t1[:])
            rstd = y2
        nm = small.tile([P, n], F32, tag=f"nmr{n}", bufs=4, name="nmr")
        nc.vector.tensor_mul(nm[:], mean_ap, rstd[:])
        nmr = small.tile([P, n], F32, tag=f"nmrn{n}", bufs=4, name="nmrn")
        nc.vector.tensor_scalar_mul(nmr[:], nm[:], -1.0)
        return rstd, nmr

    def ln_norm(out_t, y_ps, rstd, nmr, jj, s_name, b_name):
        if apply_ln_sb:
            xh = anat.tile([P, W], F32, tag="xh", bufs=2, name="xh")
            nc.scalar.activation(xh[:], y_ps[:], AF.Identity,
                                 scale=rstd[:, jj:jj + 1], bias=nmr[:, jj:jj + 1])
            xs = anat.tile([P, W], F32, tag="xh", bufs=2, name="xs")
            nc.vector.tensor_mul(xs[:], xh[:], ln_bc[s_name][:])
            nc.vector.tensor_add(out_t[:], xs[:], ln_bc[b_name][:])
        elif jj == 3:
            # every 4th normalize on DVE to relieve the ScalarE a little
            nc.vector.tensor_scalar(out_t[:], y_ps[:], rstd[:, jj:jj + 1],
                                    nmr[:, jj:jj + 1],
                                    op0=ALU.mult, op1=ALU.add)
        else:
            nc.scalar.activation(out_t[:], y_ps[:], AF.Identity,
                                 scale=rstd[:, jj:jj + 1], bias=nmr[:, jj:jj + 1])

    def out_proj_res_ln(ATs, wtiles, bias_nm, resid, s_name, b_name, out_tag,
                        pull=None):
        """Per j: y = AT^T W + resid (+bias); LN -> 4 x [P, W] bf16 tiles.
        The four y tiles live on S(2)+AB(2) so one rsqrt chain serves all
        four; `pull` emits filler (prev-pair FFN / next-pair stageA) into
        the stats->rsqrt->norm window."""
        pull = pull or (lambda n=1: None)
        outs = []
        ys = []
        mvall = small.tile([P, 8], F32, tag="mvall8", bufs=4, name="mvall")
        for j in range(4):
            pool, ptag = (psS, "S") if j < 2 else (psAB, "AB")
            ps = pool.tile([P, W], F32, tag=ptag, name="ps_y")
            for a in range(2):
                nc.tensor.matmul(ps[:], ATs[a][:, :, j * P:(j + 1) * P],
                                 wtiles[a][:],
                                 start=(a == 0), stop=False, perf_mode=DR)
            nc.tensor.matmul(ps[:], ident_sc[:], resid[j][:],
                             start=False, stop=not apply_bias)
            if apply_bias:
                nc.tensor.matmul(ps[:], ones_row[:1, :], bias_rows[bias_nm][:1, :],
                                 start=False, stop=True)
            ln_stats(ps, mvall, j)
            ys.append(ps)
            if j == 1 or j == 3:
                pull(1)
        rstd, nmr = ln_rstd(mvall, 4)
        pull(4)
        for j in range(4):
            o = anat.tile([P, W], BF16, tag=out_tag, bufs=4, name="onat")
            ln_norm(o, ys[j], rstd, nmr, j, s_name, b_name)
            outs.append(o)
        pull(2)
        return outs

    def stageA_chunks(p, x_nat, enc_nat):
        """Returns (chunks, st): closures that emit stageA work piecewise so
        they can fill the previous pair's LN2/FFN windows."""
        st = {'x_nat': x_nat}
        chunks = []

        def tr(key, srcs, tag):
            def c():
                st[key] = transpose4(srcs, tag, 4)
            return c

        def pj(key, fn):
            def c():
                st[key] = fn()
            return c

        chunks.append(tr('xT', x_nat, "xT"))
        chunks.append(tr('encT', enc_nat, "encT"))
        chunks.append(pj('QT', lambda: proj_T(mqw, st['xT'], "qt", 4)))
        chunks.append(pj('KT', lambda: proj_T(mkw, st['xT'], "kt", 4)))
        chunks.append(pj('Vn', lambda: proj_N(mvw, st['xT'], "vn", 4)))
        chunks.append(pj('KcT', lambda: proj_T(
            ckw, st['encT'], "kct", 8,
            bias_col=bias_cols['ck_b'] if apply_bias else None)))
        chunks.append(pj('VcN', lambda: proj_N(
            cvw, st['encT'], "vc", 8,
            bias_row=bias_rows['cv_b'] if apply_bias else None)))
        return chunks, st

    def ffn_chunks(p, x2T, x2):
        """Closure list for pair p's FFN: 16 f1 chunks, 4 f2 chains, 2 LN
        tails.  All PSUM on the work ring; pulled as filler during pair p+1's
        attention-output LN windows."""
        h_sbs = [None] * 8
        ln_state = {}

        def f1c(k):
            def c():
                h_ps = psW.tile([P, W], F32, tag="work", name="h_ps")
                for g in range(2):
                    nc.tensor.matmul(h_ps[:], f1sb[g][:, :, k * P:(k + 1) * P],
                                     x2T[g][:], start=(g == 0), stop=(g == 1),
                                     perf_mode=DR)
                if k % 2 == 0:
                    h_sbs[k // 2] = attn.tile([P, 2, W], FP8, tag=f"hsb_{k//2}",
                                              bufs=1, name="hsb")
                h_sb = h_sbs[k // 2]
                if k % 2 == 0:
                    nc.scalar.activation(h_sb[:, 0, :], h_ps[:], AF.Relu,
                                         bias=f1b_col[:, k:k + 1])
                else:
                    nc.vector.tensor_scalar(h_sb[:, 1, :], h_ps[:],
                                            f1b_col[:, k:k + 1],
                                            0.0, op0=ALU.add, op1=ALU.max)
            return c

        def f2c(j):
            def c():
                psF = psW.tile([P, W], F32, tag="work", name="psF")
                for kk in range(8):
                    nc.tensor.matmul(psF[:], h_sbs[kk][:, :, j * P:(j + 1) * P],
                                     f2sb[kk][:], start=(kk == 0), stop=False,
                                     perf_mode=DR)
                nc.tensor.matmul(psF[:], ident_sc[:], x2[j][:],
                                 start=False, stop=not apply_bias)
                if apply_bias:
                    nc.tensor.matmul(psF[:], ones_row[:1, :],
                                     bias_rows['f2_b'][:1, :],
                                     start=False, stop=True)
                if j % 2 == 0:
                    ln_state['mvall'] = small.tile([P, 4], F32, tag="mvall",
                                                   bufs=4, name="mvall")
                ln_stats(psF, ln_state['mvall'], j % 2)
                ln_state[j] = psF
            return c

        def tail(jh):
            def c():
                rstd, nmr = ln_rstd(ln_state['mvall'], 2)
                for jl in range(2):
                    j = 2 * jh + jl
                    o = anat.tile([P, W], F32, tag="o_nat", bufs=2, name="onat")
                    ln_norm(o, ln_state[j], rstd, nmr, jl, 'ln3_s', 'ln3_b')
                    b, th = 2 * p + j // 2, j % 2
                    nc.sync.dma_start(out=io['out'][b, th * P:(th + 1) * P, :],
                                      in_=o[:])
            return c

        return ([f1c(k) for k in range(16)]
                + [f2c(0), f2c(1), tail(0), f2c(2), f2c(3), tail(1)])

    def make_pull(chunks):
        it = iter(chunks)

        def pull(n=1):
            for _ in range(n):
                c = next(it, None)
                if c is None:
                    return
                c()
        return pull

    def stageBCD(p, st, pull):
        ATs = attention(st['QT'], st['KT'], st['Vn'], True, "p_self", pull=pull)
        x1 = out_proj_res_ln(ATs, mpw, 'mproj_b', st['x_nat'],
                             'ln1_s', 'ln1_b', "x1_nat", pull=pull)
        x1T = transpose4(x1, "x1T", 4)
        QcT = proj_T(cqw, x1T, "qct", 4,
                     bias_col=bias_cols['cq_b'] if apply_bias else None,
                     pool=(psS, "S"))
        ATc = attention(QcT, st['KcT'], st['VcN'], False, "p_cross", pull=pull)
        x2 = out_proj_res_ln(ATc, cow, 'co_b', x1, 'ln2_s', 'ln2_b', "x2_nat",
                             pull=pull)
        x2T = transpose4(x2, "x2T", 4)
        pull(40)  # drain: finish prev-pair FFN + next-pair stageA
        return ffn_chunks(p, x2T, x2)

    chunks0, st0 = stageA_chunks(0, *dma0)
    make_pull(chunks0)(13)
    sts = {0: st0}
    ffn = []
    total_pairs = n_pair * n_rep   # n_rep>1: bench-only steady-state repeat
    for pp in range(total_pairs):
        p = pp % n_pair
        filler = list(ffn)
        if pp + 1 < total_pairs:
            d_in = dma1 if pp == 0 else emit_dma_in((pp + 1) % n_pair)
            nchunks, nst = stageA_chunks((pp + 1) % n_pair, d_in[0], d_in[1])
            sts[pp + 1] = nst
            filler += nchunks
        ffn = stageBCD(p, sts.pop(pp), make_pull(filler))
    make_pull(ffn)(22)  # epilogue: last pair's FFN


_CACHE = {}


def _get_program(n_batch, apply_ln_sb, apply_bias, n_rep=1):
    key = (n_batch, apply_ln_sb, apply_bias, n_rep)
    if key not in _CACHE:
        _CACHE[key] = build_program(n_batch, apply_ln_sb, apply_bias, n_rep)
    return _CACHE[key]


def _pack_dr(w):
    """[K, N] f32 -> fp8 DoubleRow pack [K//256, 128, 2, N] at scale SW."""
    w8 = np.clip(w * SW, -240.0, 240.0).astype(FP8_NP)
    K, N = w8.shape
    return np.ascontiguousarray(
        w8.reshape(K // 256, 2, P, N).transpose(0, 2, 1, 3))


def prep_args(args):
    """Normalize + host-precision-prep the full input dict.

    2D weights are scaled by SW, cast to fp8e4 and packed for DoubleRow
    matmuls; x/enc_out are pre-cast to bf16; biases are pre-scaled to match
    the fp8 product scales (LN is scale-invariant so outputs are unchanged).
    Returns (args, apply_ln_sb, apply_bias)."""
    args = {k: np.ascontiguousarray(np.asarray(v, dtype=np.float32))
            for k, v in args.items()}

    apply_ln_sb = not all(
        (np.all(args[s] == 1.0) and np.all(args[bn] == 0.0))
        for s, bn in (('ln1_s', 'ln1_b'), ('ln2_s', 'ln2_b'), ('ln3_s', 'ln3_b')))
    apply_bias = not all(
        np.all(args[bn] == 0.0)
        for bn in ('mproj_b', 'cq_b', 'ck_b', 'cv_b', 'co_b', 'f2_b'))
    # f1_b is applied unconditionally (fused into the relu).

    for k in ('x', 'enc_out'):
        args[k] = args[k].astype(ml_dtypes.bfloat16)
    for k in ('mq_w', 'mk_w', 'mv_w'):
        args[k] = _pack_dr(args[k].reshape(E, H * D))
    for k in ('mproj_w', 'cq_w', 'ck_w', 'cv_w', 'co_w', 'f1_w', 'f2_w'):
        args[k] = _pack_dr(args[k])
    # biases enter PSUMs that carry fp8 product scales
    args['f1_b'] = args['f1_b'] * SW
    for k in ('cq_b', 'ck_b', 'cv_b'):
        args[k] = args[k] * SW
    for k in ('mproj_b', 'co_b', 'f2_b'):
        args[k] = args[k] * (SW * SW)
    return args, apply_ln_sb, apply_bias


def make_in_maps(args):
    in_maps = []
    for c in range(N_CORES):
        m = {k: args[k] for k in WEIGHT_NAMES}
        m['x'] = args['x'][c * BL:(c + 1) * BL]
        m['enc_out'] = args['enc_out'][c * BL:(c + 1) * BL]
        in_maps.append(m)
    return in_maps


def kernel(x, enc_out, mq_w, mk_w, mv_w, mproj_w, mproj_b,
           cq_w, cq_b, ck_w, ck_b, cv_w, cv_b, co_w, co_b,
           f1_w, f1_b, f2_w, f2_b,
           ln1_s, ln1_b, ln2_s, ln2_b, ln3_s, ln3_b,
           _trace=False):
    args = dict(x=x, enc_out=enc_out, mq_w=mq_w, mk_w=mk_w, mv_w=mv_w,
                mproj_w=mproj_w, mproj_b=mproj_b, cq_w=cq_w, cq_b=cq_b,
                ck_w=ck_w, ck_b=ck_b, cv_w=cv_w, cv_b=cv_b, co_w=co_w,
                co_b=co_b, f1_w=f1_w, f1_b=f1_b, f2_w=f2_w, f2_b=f2_b,
                ln1_s=ln1_s, ln1_b=ln1_b, ln2_s=ln2_s, ln2_b=ln2_b,
                ln3_s=ln3_s, ln3_b=ln3_b)
    args, apply_ln_sb, apply_bias = prep_args(args)
    nc = _get_program(BL, apply_ln_sb, apply_bias)
    in_maps = make_in_maps(args)

    res = run_bass_kernel_spmd(nc, in_maps, list(range(N_CORES)), trace=_trace)
    out = np.concatenate([res.results[c]['out'] for c in range(N_CORES)], axis=0)
    if _trace:
        kernel.last_results = res
    return out

